# revision 6
# baseline (speedup 1.0000x reference)
"""DynamicConv1D Trainium2 kernel (v2 — engine-balanced).

Reference computation (per batch b):
  dw = conv1d(x, W, pad=3) + b            # [O*I*K, T] dynamic weights
  dw = softmax(dw.reshape(O,I,K,T)/sqrt(K), axis=K)
  y[o,t] = sum_{i,k} x[i, t+k-3] * dw[o,i,k,t]

Sharding: 8 cores = 4 batches x 2 halves of O (16 out-channels each).

v2 changes vs the 284us baseline (vector engine was 82% busy):
  * softmax shift-invariance: host subtracts the k=6 tap's conv weights
    and bias from taps k=0..5, so the kernel computes only 6 exp slabs
    (e'_6 == 1). Exact algebra; PE/ACT/DVE-mul work all drop 1/7th.
      den' = 1 + sum_{k<6} e'_k
      num' = x_6 + sum_{k<6} e'_k x_k,   y[t,o] = sum_i num'/den'
  * den k-sum pair level runs on the (otherwise idle) GpSimd engine.
  * 1/den via ScalarE Ln + Exp(-x) (one act table holds both), keeping
    the whole den/num/y tail in bf16; removes the f32 reciprocal, f32
    multiply and most f32 adds from the DVE.
  * PSUM chunks are 1536 wide (3 banks): 2 exp activations per tile
    instead of 7, halving ScalarE per-op overhead.
  * pair-of-tiles batching for all tail ops to halve DVE/Pool/ACT
    instruction count.
"""

import numpy as np

B = 4
C = 32
K = 7
K6 = 6  # taps actually computed (k=6 folded out by the softmax shift)
T = 4096
O_FULL = 32
OH = 16  # out-channels per core
PAD = 3
TT = 128  # t positions per tile (partition dim)
FREE = K6 * OH * C  # 3072, free index = k*512 + o*32 + i  (k < 6)
SLAB = OH * C  # 512, one k-slab
CD1 = 128  # (j, c) rows for j=0..3
CD2 = 97  # (j, c) rows for j=4..6 plus ones row
KC = K * C  # 224 columns of x_unf (all 7 taps)
CH = 1536  # psum chunk (3 banks); FREE = 2*CH

_prog_cache = {}


def _build(t_len):
    """Build and compile the per-core Bass program for sequence length t_len."""
    import concourse.tile as tile
    from concourse import bacc, mybir

    nt = t_len // TT
    nc = bacc.Bacc("TRN2", target_bir_lowering=False, debug=False, num_devices=1)
    f32 = mybir.dt.float32
    bf16 = mybir.dt.bfloat16
    AF = mybir.ActivationFunctionType
    ADD = mybir.AluOpType.add

    x1a_d = nc.dram_tensor("x1a", [CD1, t_len], bf16, kind="ExternalInput").ap()
    x1b_d = nc.dram_tensor("x1b", [CD2, t_len], bf16, kind="ExternalInput").ap()
    w1_d = nc.dram_tensor("wp1", [CD1, FREE], bf16, kind="ExternalInput").ap()
    w2_d = nc.dram_tensor("wp2", [CD2, FREE], bf16, kind="ExternalInput").ap()
    y_d = nc.dram_tensor("yout", [TT, nt * OH], f32, kind="ExternalOutput").ap()

    with tile.TileContext(nc) as tc:
        with (
            tc.tile_pool(name="const", bufs=1) as cpool,
            tc.tile_pool(name="x2p", bufs=3) as x2pool,
            tc.tile_pool(name="ep", bufs=2) as epool,
            tc.tile_pool(name="tree", bufs=2) as tpool,
            tc.tile_pool(name="small", bufs=2) as spool,
            tc.tile_pool(name="psum", bufs=2, space="PSUM") as ppool,
        ):
            x1a_bf = cpool.tile([CD1, t_len], bf16, tag="x1abf")
            x1b_bf = cpool.tile([CD2, t_len], bf16, tag="x1bbf")
            w1_bf = cpool.tile([CD1, FREE], bf16, tag="w1bf")
            w2_bf = cpool.tile([CD2, FREE], bf16, tag="w2bf")
            y_sb = cpool.tile([TT, nt * OH], f32, tag="ysb")

            # Input loads: first-needed first; spread across the sync,
            # scalar and gpsimd DMA queues so no single queue serializes
            # the prologue.
            c0 = 4 * TT
            nc.sync.dma_start(x1a_bf[:, 0:c0], x1a_d[:, 0:c0])
            nc.scalar.dma_start(x1b_bf[:, 0:c0], x1b_d[:, 0:c0])
            nc.sync.dma_start(w1_bf[:, 0:CH], w1_d[:, 0:CH])
            nc.scalar.dma_start(w2_bf[:, 0:CH], w2_d[:, 0:CH])
            nc.gpsimd.dma_start(w1_bf[:, CH:], w1_d[:, CH:])
            nc.gpsimd.dma_start(w2_bf[:, CH:], w2_d[:, CH:])
            nc.sync.dma_start(x1a_bf[:, c0:], x1a_d[:, c0:])
            nc.scalar.dma_start(x1b_bf[:, c0:], x1b_d[:, c0:])

            for pp in range(nt // 2):
                # x2[(u), j*32+c] = x[c, t0+tp+j-3] for the tile pair; built
                # by DMA transposes of the im2col rows.
                x2p = x2pool.tile([TT, 2, KC], bf16, tag="x2")
                # eex[side, u, koi]: side 0 = e' (exp slabs), side 1 = e'*x.
                eex = epool.tile([TT, 2, 2, FREE], bf16, tag="eex")
                for u in range(2):
                    tt = 2 * pp + u
                    t0 = tt * TT
                    nc.sync.dma_start_transpose(
                        x2p[:, u, 0:CD1], x1a_bf[:, t0 : t0 + TT]
                    )
                    nc.sync.dma_start_transpose(
                        x2p[:, u, CD1:KC], x1b_bf[0 : CD2 - 1, t0 : t0 + TT]
                    )
                    for ci in range(2):
                        pc = ppool.tile([TT, CH], f32, tag="pc", name="pc")
                        # group matmuls by stationary operand (x1a then x1b)
                        for j in range(3):
                            cs = slice(ci * CH + j * 512, ci * CH + (j + 1) * 512)
                            nc.tensor.matmul(
                                pc[:, j * 512 : (j + 1) * 512],
                                x1a_bf[:, t0 : t0 + TT], w1_bf[:, cs],
                                start=True, stop=False,
                            )
                        for j in range(3):
                            cs = slice(ci * CH + j * 512, ci * CH + (j + 1) * 512)
                            nc.tensor.matmul(
                                pc[:, j * 512 : (j + 1) * 512],
                                x1b_bf[:, t0 : t0 + TT], w2_bf[:, cs],
                                start=False, stop=True,
                            )
                        nc.scalar.activation(
                            eex[:, 0, u, ci * CH : (ci + 1) * CH], pc[:], AF.Exp
                        )

                # --- pair-wide tail ---
                # EX = e * x (broadcast over o); per tile u so the AP stays
                # within 3 free dims.
                for u in range(2):
                    e4 = eex[:, 0, u].rearrange("p (k o i) -> p k o i", k=K6, o=OH)
                    ex4 = eex[:, 1, u].rearrange("p (k o i) -> p k o i", k=K6, o=OH)
                    x24 = (
                        x2p[:, u, 0 : K6 * C]
                        .rearrange("p (k i) -> p k i", k=K6)
                        .unsqueeze(2)
                        .broadcast_to([TT, K6, OH, C])
                    )
                    nc.vector.tensor_mul(ex4, e4, x24)

                # num k-sum tree (DVE): 6 ex slabs + x6.
                exu = eex[:, 1].rearrange(
                    "p s (x k q) -> p (s x) k q", x=3, k=2
                )  # [TT, 6(u,x), 2, 512]
                t1n = tpool.tile([TT, 2, 3, SLAB], bf16, tag="t1n")
                t1nv = t1n[:].rearrange("p u x q -> p (u x) q")
                nc.vector.tensor_add(t1nv, exu[:, :, 0], exu[:, :, 1])
                t2n = spool.tile([TT, 2, SLAB], bf16, tag="t2n")
                nc.vector.tensor_add(t2n[:], t1n[:, :, 0], t1n[:, :, 1])
                t3n = spool.tile([TT, 2, SLAB], bf16, tag="t3n")
                x6b = (
                    x2p[:, :, K6 * C : KC]
                    .unsqueeze(2)
                    .broadcast_to([TT, 2, OH, C])
                )
                nc.vector.tensor_add(
                    t3n[:].rearrange("p u (o i) -> p u o i", o=OH),
                    t1n[:, :, 2].rearrange("p u (o i) -> p u o i", o=OH),
                    x6b,
                )
                dn2 = spool.tile([TT, 2, SLAB], bf16, tag="dn2")
                nc.vector.tensor_add(dn2[:], t2n[:], t3n[:])

                # den k-sum pair level on GpSimd (one wide op).
                eu = eex[:, 0].rearrange("p s (x k q) -> p (s x) k q", x=3, k=2)
                t1d = tpool.tile([TT, 2, 3, SLAB], bf16, tag="t1d")
                t1dv = t1d[:].rearrange("p u x q -> p (u x) q")
                nc.gpsimd.tensor_add(t1dv, eu[:, :, 0], eu[:, :, 1])
                t2d = spool.tile([TT, 2, SLAB], bf16, tag="t2d")
                nc.vector.tensor_add(t2d[:], t1d[:, :, 0], t1d[:, :, 1])
                # den = (t1d2 + 1) + t2d  (the +1 is e'_6)
                denb = spool.tile([TT, 2, SLAB], bf16, tag="denb")
                nc.vector.scalar_tensor_tensor(
                    denb[:], t1d[:, :, 2], 1.0, t2d[:], op0=ADD, op1=ADD
                )

                # 1/den on ScalarE: exp(-ln(den)).
                lden = spool.tile([TT, 2, SLAB], bf16, tag="lden")
                nc.scalar.activation(lden[:], denb[:], AF.Ln)
                rr = spool.tile([TT, 2, SLAB], bf16, tag="rr")
                nc.scalar.activation(rr[:], lden[:], AF.Exp, scale=-1.0)

                # y[t,o] = sum_i num * (1/den)
                y1 = spool.tile([TT, 2, SLAB], bf16, tag="y1")
                nc.vector.tensor_mul(y1[:], dn2[:], rr[:])
                nc.vector.tensor_reduce(
                    y_sb[:, pp * 2 * OH : (pp + 1) * 2 * OH],
                    y1[:].rearrange("p u (o i) -> p u o i", o=OH),
                    axis=mybir.AxisListType.X,
                    op=ADD,
                )

                if (pp + 1) % 4 == 0 or pp == nt // 2 - 1:
                    g0 = (pp // 4) * 8 * OH
                    nc.sync.dma_start(
                        y_d[:, g0 : (pp + 1) * 2 * OH], y_sb[:, g0 : (pp + 1) * 2 * OH]
                    )

    # Both Exp and Ln live in the natural_log_exp_and_others act table, but
    # the table-load pass picks the first set containing each function,
    # alternating tables (a 1.3us reload per switch). Trim exp/ln from every
    # other set (indices into act_info.json are preserved) so one load serves
    # the whole kernel.
    from concourse.hw_specs import get_activation_tables

    tabs = get_activation_tables(nc.m.arch)
    for name, s in tabs.items():
        if name != "natural_log_exp_and_others":
            s.discard(mybir.ActivationFunctionType.Exp)
            s.discard(mybir.ActivationFunctionType.Ln)

    nc.compile()
    return nc


def _prep_inputs(x, W, b):
    """Host-side scatter: per-core input dicts (layout + the k=6 shift)."""
    import ml_dtypes

    bf = ml_dtypes.bfloat16
    scale = np.float32(1.0 / np.sqrt(K))
    halves = []
    for h in range(2):
        Wh = W[h * OH * C * K : (h + 1) * OH * C * K]  # rows (o, i, k)
        W5 = Wh.reshape(OH, C, K, C, K)  # (o, i, k, c, j)
        # softmax shift: subtract the k=6 tap, drop it
        Ws = W5[:, :, :K6] - W5[:, :, K6 : K6 + 1]
        # rows (j,c) -> j*32+c ; cols (k,o,i) -> k*512 + o*32 + i, k<6
        Wp = Ws.transpose(4, 3, 2, 0, 1).reshape(K * C, FREE) * scale
        bh = b[h * OH * C * K : (h + 1) * OH * C * K].reshape(OH, C, K)
        bs = (bh[:, :, :K6] - bh[:, :, K6 : K6 + 1]) * scale
        bs = bs.transpose(2, 0, 1).reshape(FREE)
        w1 = np.ascontiguousarray(Wp[:CD1])
        w2 = np.ascontiguousarray(np.concatenate([Wp[CD1:], bs[None, :]], axis=0))
        halves.append((w1.astype(bf), w2.astype(bf)))

    t_len = x.shape[-1]
    x1s = []
    for bi in range(B):
        xp = np.zeros((C, t_len + 2 * PAD), dtype=np.float32)
        xp[:, PAD : PAD + t_len] = x[bi]
        x1a = np.empty((CD1, t_len), dtype=np.float32)
        x1b = np.empty((CD2, t_len), dtype=np.float32)
        for j in range(K):
            tgt, r0 = (x1a, j * C) if j < 4 else (x1b, (j - 4) * C)
            tgt[r0 : r0 + C] = xp[:, j : j + t_len]
        x1b[CD2 - 1] = 1.0
        x1s.append((x1a.astype(bf), x1b.astype(bf)))

    in_maps = []
    for core in range(8):
        bi, h = divmod(core, 2)
        w1, w2 = halves[h]
        x1a, x1b = x1s[bi]
        in_maps.append({"x1a": x1a, "x1b": x1b, "wp1": w1, "wp2": w2})
    return in_maps


def _assemble(results, t_len):
    """Gather per-core [TT, nt*OH] outputs into [B, O_FULL, t_len]."""
    nt = t_len // TT
    y = np.empty((B, O_FULL, t_len), dtype=np.float32)
    for core, res in enumerate(results):
        bi, h = divmod(core, 2)
        arr = res["yout"].reshape(TT, nt, OH)  # [tp, tt, o]
        y[bi, h * OH : (h + 1) * OH, :] = arr.transpose(2, 1, 0).reshape(OH, t_len)
    return y


def _run(x, W, b, trace=False, trace_cores=None):
    from concourse.bass_utils import run_bass_kernel_spmd
    from concourse.bass_interp import get_hw_module

    t_len = x.shape[-1]
    key = ("prog", t_len)
    if key not in _prog_cache:
        nc = _build(t_len)
        nc.m = get_hw_module(nc.m)
        _prog_cache[key] = nc
    nc = _prog_cache[key]

    in_maps = _prep_inputs(x, W, b)
    res = run_bass_kernel_spmd(
        nc,
        in_maps,
        core_ids=list(range(8)),
        trace=trace,
        trace_cores=trace_cores,
    )
    return _assemble(res.results, t_len), res


def kernel(x, W, b):
    y, _ = _run(np.asarray(x), np.asarray(W), np.asarray(b))
    return y


# revision 8
# speedup vs baseline: 1.3447x; 1.3447x over previous
"""DynamicConv1D Trainium2 kernel (v3).

Reference computation (per batch b):
  dw = conv1d(x, W, pad=3) + b            # [O*I*K, T] dynamic weights
  dw = softmax(dw.reshape(O,I,K,T)/sqrt(K), axis=K)
  y[o,t] = sum_{i,k} x[i, t+k-3] * dw[o,i,k,t]

Sharding: 8 cores = 4 batches x 2 halves of O (16 out-channels each).

Key optimizations vs the 284us baseline (vector engine was 82% busy):
  * softmax shift-invariance: host subtracts the k=6 tap's conv weights
    and bias from taps k=0..5, so the kernel computes only 6 exp slabs
    (e'_6 == 1). Exact algebra; PE/ACT/DVE-mul work all drop 1/7th.
      den' = 1 + sum_{k<6} e'_k
      num' = x_6 + sum_{k<6} e'_k x_k,   y[t,o] = sum_i num'/den'
  * 1/den via ScalarE Ln + Exp(-x); the +1 of den' rides the Ln bias.
    Whole den/num/y tail stays bf16 (2x DVE) — no f32 reciprocal chain.
  * PSUM chunks 1536 wide: 2 exp activations per tile instead of 7.
  * quad-of-tiles batching for all tail ops (4x fewer instructions).
  * all elementwise work stays on DVE: offloading to GpSimd was tried
    and hurts — Pool and DVE share SBUF ports, concurrent tensor ops
    stretch DVE ~4x.
"""

import numpy as np

B = 4
C = 32
K = 7
K6 = 6  # taps actually computed (k=6 folded out by the softmax shift)
T = 4096
O_FULL = 32
OH = 16  # out-channels per core
PAD = 3
TT = 128  # t positions per tile (partition dim)
FREE = K6 * OH * C  # 3072, free index = k*512 + o*32 + i  (k < 6)
SLAB = OH * C  # 512, one k-slab
CD1 = 128  # (j, c) rows for j=0..3
CD2 = 97  # (j, c) rows for j=4..6 plus ones row
KC = K * C  # 224 columns of x_unf (all 7 taps)
CH = 1536  # psum chunk (3 banks); FREE = 2*CH
QU = 4  # tiles per batching group

_prog_cache = {}


def _build(t_len):
    """Build and compile the per-core Bass program for sequence length t_len."""
    import concourse.tile as tile
    from concourse import bacc, mybir

    nt = t_len // TT
    nc = bacc.Bacc("TRN2", target_bir_lowering=False, debug=False, num_devices=1)
    f32 = mybir.dt.float32
    bf16 = mybir.dt.bfloat16
    AF = mybir.ActivationFunctionType
    ADD = mybir.AluOpType.add

    x1a_d = nc.dram_tensor("x1a", [CD1, t_len], bf16, kind="ExternalInput").ap()
    x1b_d = nc.dram_tensor("x1b", [CD2, t_len], bf16, kind="ExternalInput").ap()
    w1_d = nc.dram_tensor("wp1", [CD1, FREE], bf16, kind="ExternalInput").ap()
    w2_d = nc.dram_tensor("wp2", [CD2, FREE], bf16, kind="ExternalInput").ap()
    y_d = nc.dram_tensor("yout", [TT, nt * OH], f32, kind="ExternalOutput").ap()

    with tile.TileContext(nc) as tc:
        with (
            tc.tile_pool(name="const", bufs=1) as cpool,
            tc.tile_pool(name="x2p", bufs=3) as x2pool,
            tc.tile_pool(name="ep", bufs=2) as epool,
            tc.tile_pool(name="tree", bufs=1) as tpool,
            tc.tile_pool(name="small", bufs=1) as spool,
            tc.tile_pool(name="hand", bufs=2) as hpool,
            tc.tile_pool(name="psum", bufs=2, space="PSUM") as ppool,
        ):
            x1a_bf = cpool.tile([CD1, t_len], bf16, tag="x1abf")
            x1b_bf = cpool.tile([CD2, t_len], bf16, tag="x1bbf")
            w1_bf = cpool.tile([CD1, FREE], bf16, tag="w1bf")
            w2_bf = cpool.tile([CD2, FREE], bf16, tag="w2bf")
            y_sb = cpool.tile([TT, nt * OH], f32, tag="ysb")

            # Input loads, first-needed first, split across the sync and
            # gpsimd DMA queues (gpsimd SWDGE issue is cheap; scalar-queue
            # DMAs were measured costing ACT ~8us each in issue time).
            c0 = QU * TT
            nc.sync.dma_start(x1a_bf[:, 0:c0], x1a_d[:, 0:c0])
            nc.gpsimd.dma_start(x1b_bf[:, 0:c0], x1b_d[:, 0:c0])
            nc.sync.dma_start(w1_bf[:, 0:CH], w1_d[:, 0:CH])
            nc.gpsimd.dma_start(w2_bf[:, 0:CH], w2_d[:, 0:CH])
            nc.sync.dma_start(w1_bf[:, CH:], w1_d[:, CH:])
            nc.gpsimd.dma_start(w2_bf[:, CH:], w2_d[:, CH:])
            nc.sync.dma_start(x1a_bf[:, c0:], x1a_d[:, c0:])
            nc.gpsimd.dma_start(x1b_bf[:, c0:], x1b_d[:, c0:])

            for pp in range(nt // QU):
                # x2[u, j*32+c] = x[c, t0+tp+j-3] for the tile quad, via DMA
                # transposes of the im2col rows.
                x2p = x2pool.tile([TT, QU, KC], bf16, tag="x2")
                # eex[side, u, koi]: side 0 = e' (exp slabs), side 1 = e'*x.
                eex = epool.tile([TT, 2, QU, FREE], bf16, tag="eex")
                for u in range(QU):
                    tt = QU * pp + u
                    t0 = tt * TT
                    nc.sync.dma_start_transpose(
                        x2p[:, u, 0:CD1], x1a_bf[:, t0 : t0 + TT]
                    )
                    nc.sync.dma_start_transpose(
                        x2p[:, u, CD1:KC], x1b_bf[0 : CD2 - 1, t0 : t0 + TT]
                    )
                    for ci in range(2):
                        pc = ppool.tile([TT, CH], f32, tag="pc", name="pc")
                        # group matmuls by stationary operand (x1a then x1b)
                        for j in range(3):
                            cs = slice(ci * CH + j * 512, ci * CH + (j + 1) * 512)
                            nc.tensor.matmul(
                                pc[:, j * 512 : (j + 1) * 512],
                                x1a_bf[:, t0 : t0 + TT], w1_bf[:, cs],
                                start=True, stop=False,
                            )
                        for j in range(3):
                            cs = slice(ci * CH + j * 512, ci * CH + (j + 1) * 512)
                            nc.tensor.matmul(
                                pc[:, j * 512 : (j + 1) * 512],
                                x1b_bf[:, t0 : t0 + TT], w2_bf[:, cs],
                                start=False, stop=True,
                            )
                        nc.scalar.activation(
                            eex[:, 0, u, ci * CH : (ci + 1) * CH], pc[:], AF.Exp
                        )
                    # EX = e * x (broadcast over o); per tile so the AP
                    # stays within 3 free dims.
                    e4 = eex[:, 0, u].rearrange("p (k o i) -> p k o i", k=K6, o=OH)
                    ex4 = eex[:, 1, u].rearrange("p (k o i) -> p k o i", k=K6, o=OH)
                    x24 = (
                        x2p[:, u, 0 : K6 * C]
                        .rearrange("p (k i) -> p k i", k=K6)
                        .unsqueeze(2)
                        .broadcast_to([TT, K6, OH, C])
                    )
                    nc.vector.tensor_mul(ex4, e4, x24)

                # --- quad-wide tail (all DVE ops bf16 @2x) ---
                # num k-sum tree: 6 ex slabs + x6.
                exu = eex[:, 1].rearrange(
                    "p s (x k q) -> p (s x) k q", x=3, k=2
                )  # [TT, 12(u,x), 2, 512]
                t1n = tpool.tile([TT, QU, 3, SLAB], bf16, tag="t1n")
                t1nv = t1n[:].rearrange("p u x q -> p (u x) q")
                nc.vector.tensor_add(t1nv, exu[:, :, 0], exu[:, :, 1])
                t2n = spool.tile([TT, QU, SLAB], bf16, tag="t2n")
                nc.vector.tensor_add(t2n[:], t1n[:, :, 0], t1n[:, :, 1])
                t3n = spool.tile([TT, QU, SLAB], bf16, tag="t3n")
                x6b = (
                    x2p[:, :, K6 * C : KC]
                    .unsqueeze(2)
                    .broadcast_to([TT, QU, OH, C])
                )
                nc.vector.tensor_add(
                    t3n[:].rearrange("p u (o i) -> p u o i", o=OH),
                    t1n[:, :, 2].rearrange("p u (o i) -> p u o i", o=OH),
                    x6b,
                )
                dn2 = spool.tile([TT, QU, SLAB], bf16, tag="dn2")
                nc.vector.tensor_add(dn2[:], t2n[:], t3n[:])

                # den k-sum tree (no +1: it rides the Ln bias).
                eu = eex[:, 0].rearrange("p s (x k q) -> p (s x) k q", x=3, k=2)
                t1d = tpool.tile([TT, QU, 3, SLAB], bf16, tag="t1d")
                t1dv = t1d[:].rearrange("p u x q -> p (u x) q")
                nc.vector.tensor_add(t1dv, eu[:, :, 0], eu[:, :, 1])
                t2d = spool.tile([TT, QU, SLAB], bf16, tag="t2d")
                nc.vector.tensor_add(t2d[:], t1d[:, :, 0], t1d[:, :, 1])
                denb = hpool.tile([TT, QU, SLAB], bf16, tag="denb")
                nc.vector.tensor_add(denb[:], t1d[:, :, 2], t2d[:])

                # 1/den on ScalarE: exp(-ln(den + 1)).
                lden = hpool.tile([TT, QU, SLAB], bf16, tag="lden")
                nc.scalar.activation(lden[:], denb[:], AF.Ln, bias=1.0)
                rr = hpool.tile([TT, QU, SLAB], bf16, tag="rr")
                nc.scalar.activation(rr[:], lden[:], AF.Exp, scale=-1.0)

                # y[t,o] = sum_i num * (1/den): mul, halve over i, reduce.
                y1 = spool.tile([TT, QU, SLAB], bf16, tag="y1")
                nc.vector.tensor_mul(y1[:], dn2[:], rr[:])
                yh = spool.tile([TT, QU * OH, C // 2], bf16, tag="yh")
                y1h = y1[:].rearrange("p u (o h i) -> p (u o) h i", o=OH, h=2)
                nc.vector.tensor_add(yh[:], y1h[:, :, 0], y1h[:, :, 1])
                nc.vector.tensor_reduce(
                    y_sb[:, pp * QU * OH : (pp + 1) * QU * OH],
                    yh[:],
                    axis=mybir.AxisListType.X,
                    op=ADD,
                )

                if (pp + 1) % 2 == 0 or pp == nt // QU - 1:
                    g0 = (pp // 2) * 2 * QU * OH
                    nc.sync.dma_start(
                        y_d[:, g0 : (pp + 1) * QU * OH],
                        y_sb[:, g0 : (pp + 1) * QU * OH],
                    )

    # Both Exp and Ln live in the natural_log_exp_and_others act table, but
    # the table-load pass picks the first set containing each function,
    # alternating tables (a 1.3us reload per switch). Trim exp/ln from every
    # other set (indices into act_info.json are preserved) so one load serves
    # the whole kernel.
    from concourse.hw_specs import get_activation_tables

    tabs = get_activation_tables(nc.m.arch)
    for name, s in tabs.items():
        if name != "natural_log_exp_and_others":
            s.discard(mybir.ActivationFunctionType.Exp)
            s.discard(mybir.ActivationFunctionType.Ln)

    nc.compile()
    return nc


def _prep_inputs(x, W, b):
    """Host-side scatter: per-core input dicts (layout + the k=6 shift)."""
    import ml_dtypes

    bf = ml_dtypes.bfloat16
    scale = np.float32(1.0 / np.sqrt(K))
    halves = []
    for h in range(2):
        Wh = W[h * OH * C * K : (h + 1) * OH * C * K]  # rows (o, i, k)
        W5 = Wh.reshape(OH, C, K, C, K)  # (o, i, k, c, j)
        # softmax shift: subtract the k=6 tap, drop it
        Ws = W5[:, :, :K6] - W5[:, :, K6 : K6 + 1]
        # rows (j,c) -> j*32+c ; cols (k,o,i) -> k*512 + o*32 + i, k<6
        Wp = Ws.transpose(4, 3, 2, 0, 1).reshape(K * C, FREE) * scale
        bh = b[h * OH * C * K : (h + 1) * OH * C * K].reshape(OH, C, K)
        bs = (bh[:, :, :K6] - bh[:, :, K6 : K6 + 1]) * scale
        bs = bs.transpose(2, 0, 1).reshape(FREE)
        w1 = np.ascontiguousarray(Wp[:CD1])
        w2 = np.ascontiguousarray(np.concatenate([Wp[CD1:], bs[None, :]], axis=0))
        halves.append((w1.astype(bf), w2.astype(bf)))

    t_len = x.shape[-1]
    x1s = []
    for bi in range(B):
        xp = np.zeros((C, t_len + 2 * PAD), dtype=np.float32)
        xp[:, PAD : PAD + t_len] = x[bi]
        x1a = np.empty((CD1, t_len), dtype=np.float32)
        x1b = np.empty((CD2, t_len), dtype=np.float32)
        for j in range(K):
            tgt, r0 = (x1a, j * C) if j < 4 else (x1b, (j - 4) * C)
            tgt[r0 : r0 + C] = xp[:, j : j + t_len]
        x1b[CD2 - 1] = 1.0
        x1s.append((x1a.astype(bf), x1b.astype(bf)))

    in_maps = []
    for core in range(8):
        bi, h = divmod(core, 2)
        w1, w2 = halves[h]
        x1a, x1b = x1s[bi]
        in_maps.append({"x1a": x1a, "x1b": x1b, "wp1": w1, "wp2": w2})
    return in_maps


def _assemble(results, t_len):
    """Gather per-core [TT, nt*OH] outputs into [B, O_FULL, t_len]."""
    nt = t_len // TT
    y = np.empty((B, O_FULL, t_len), dtype=np.float32)
    for core, res in enumerate(results):
        bi, h = divmod(core, 2)
        arr = res["yout"].reshape(TT, nt, OH)  # [tp, tt, o]
        y[bi, h * OH : (h + 1) * OH, :] = arr.transpose(2, 1, 0).reshape(OH, t_len)
    return y


def _run(x, W, b, trace=False, trace_cores=None):
    from concourse.bass_utils import run_bass_kernel_spmd
    from concourse.bass_interp import get_hw_module

    t_len = x.shape[-1]
    key = ("prog", t_len)
    if key not in _prog_cache:
        nc = _build(t_len)
        nc.m = get_hw_module(nc.m)
        _prog_cache[key] = nc
    nc = _prog_cache[key]

    in_maps = _prep_inputs(x, W, b)
    res = run_bass_kernel_spmd(
        nc,
        in_maps,
        core_ids=list(range(8)),
        trace=trace,
        trace_cores=trace_cores,
    )
    return _assemble(res.results, t_len), res


def kernel(x, W, b):
    y, _ = _run(np.asarray(x), np.asarray(W), np.asarray(b))
    return y


# revision 9
# speedup vs baseline: 1.4609x; 1.0864x over previous
"""DynamicConv1D Trainium2 kernel (v4).

Reference computation (per batch b):
  dw = conv1d(x, W, pad=3) + b            # [O*I*K, T] dynamic weights
  dw = softmax(dw.reshape(O,I,K,T)/sqrt(K), axis=K)
  y[o,t] = sum_{i,k} x[i, t+k-3] * dw[o,i,k,t]

Sharding: 8 cores = 4 batches x 2 halves of O (16 out-channels each).

Optimizations vs the 284us baseline:
  * softmax shift-invariance: host subtracts the k=6 tap's conv weights
    and bias from taps k=0..5, so only 6 exp slabs are computed
    (e'_6 == 1). Exact algebra; PE/ACT/DVE work all drop 1/7th.
      den' = 1 + sum_{k<6} e'_k
      num' = x_6 + sum_{k<6} e'_k x_k,   y[t,o] = sum_i num'/den'
  * conv as fp8(e4m3) DoubleRow matmuls: both 128-row contraction
    halves (im2col taps 0-3 | taps 4-6 + bias row) ride one matmul at
    0.5 cycles/row. Weights are pre-scaled by 16 to dodge fp8
    subnormals; the 1/16 rides the exp activation's scale. (~1% rel
    err, tolerance is 2e-2.)
  * x_unf (x2) is built host-side and DMA'd, replacing 64 DMA
    transposes that serialized behind bulk loads on the sync queue.
  * 1/den via ScalarE Ln + Exp(-x); the +1 of den' rides the Ln bias.
    The whole den/num/y tail stays bf16 (2x DVE rate).
  * quad-of-tiles batching for tail ops; 1536-wide psum chunks (2 exp
    activations per tile instead of 7).
  * all elementwise work stays on DVE: GpSimd offload was tried and
    hurts (Pool and DVE share SBUF ports; concurrent tensor ops
    stretch DVE ~4x).
"""

import numpy as np

B = 4
C = 32
K = 7
K6 = 6  # taps actually computed (k=6 folded out by the softmax shift)
T = 4096
O_FULL = 32
OH = 16  # out-channels per core
PAD = 3
TT = 128  # t positions per tile (partition dim)
FREE = K6 * OH * C  # 3072, free index = k*512 + o*32 + i  (k < 6)
SLAB = OH * C  # 512, one k-slab
CD1 = 128
KC = K * C  # 224 columns of x_unf (all 7 taps)
CH = 1536  # psum chunk (3 banks); FREE = 2*CH
QU = 4  # tiles per batching group
WSC = 16.0  # fp8 weight pre-scale (1/WSC folded into the exp activation)

_prog_cache = {}


def _build(t_len):
    """Build and compile the per-core Bass program for sequence length t_len."""
    import concourse.tile as tile
    from concourse import bacc, mybir

    nt = t_len // TT
    nc = bacc.Bacc("TRN2", target_bir_lowering=False, debug=False, num_devices=1)
    f32 = mybir.dt.float32
    bf16 = mybir.dt.bfloat16
    fp8 = mybir.dt.float8e4
    AF = mybir.ActivationFunctionType
    ADD = mybir.AluOpType.add
    DR = mybir.MatmulPerfMode.DoubleRow

    x1_d = nc.dram_tensor("x1f8", [CD1, 2, t_len], fp8, kind="ExternalInput").ap()
    w_d = nc.dram_tensor("w8", [CD1, 2, FREE], fp8, kind="ExternalInput").ap()
    x2_d = nc.dram_tensor("x2f", [TT, nt * KC], bf16, kind="ExternalInput").ap()
    y_d = nc.dram_tensor("yout", [TT, nt * OH], f32, kind="ExternalOutput").ap()

    with tile.TileContext(nc) as tc:
        with (
            tc.tile_pool(name="const", bufs=1) as cpool,
            tc.tile_pool(name="ep", bufs=2) as epool,
            tc.tile_pool(name="tree", bufs=1) as tpool,
            tc.tile_pool(name="small", bufs=1) as spool,
            tc.tile_pool(name="hand", bufs=2) as hpool,
            tc.tile_pool(name="psum", bufs=2, space="PSUM") as ppool,
        ):
            x1_sb = cpool.tile([CD1, 2, t_len], fp8, tag="x1sb")
            w_sb = cpool.tile([CD1, 2, FREE], fp8, tag="wsb")
            x2_sb = cpool.tile([TT, nt * KC], bf16, tag="x2sb")
            y_sb = cpool.tile([TT, nt * OH], f32, tag="ysb")

            # Input loads, first-needed first. x1/w on the gpsimd (SWDGE)
            # queue; x2 in per-double-quad slices on sync so early quads
            # unblock while the tail streams.
            c0 = QU * TT
            nc.gpsimd.dma_start(x1_sb[:, :, 0:c0], x1_d[:, :, 0:c0])
            nc.gpsimd.dma_start(w_sb[:], w_d[:])
            nc.gpsimd.dma_start(x1_sb[:, :, c0:], x1_d[:, :, c0:])
            xsl = 2 * QU * KC
            for g in range(nt // (2 * QU)):
                nc.sync.dma_start(
                    x2_sb[:, g * xsl : (g + 1) * xsl], x2_d[:, g * xsl : (g + 1) * xsl]
                )

            for pp in range(nt // QU):
                # eex[side, u, koi]: side 0 = e' (exp slabs), side 1 = e'*x.
                eex = epool.tile([TT, 2, QU, FREE], bf16, tag="eex")
                for u in range(QU):
                    tt = QU * pp + u
                    t0 = tt * TT
                    for ci in range(2):
                        pc = ppool.tile([TT, CH], f32, tag="pc", name="pc")
                        for j in range(3):
                            cs = slice(ci * CH + j * 512, ci * CH + (j + 1) * 512)
                            nc.tensor.matmul(
                                pc[:, j * 512 : (j + 1) * 512],
                                x1_sb[:, :, t0 : t0 + TT],
                                w_sb[:, :, cs],
                                start=True, stop=True,
                                perf_mode=DR,
                            )
                        nc.scalar.activation(
                            eex[:, 0, u, ci * CH : (ci + 1) * CH], pc[:], AF.Exp,
                            scale=1.0 / WSC,
                        )
                    # EX = e * x (broadcast over o); per tile so the AP
                    # stays within 3 free dims.
                    e4 = eex[:, 0, u].rearrange("p (k o i) -> p k o i", k=K6, o=OH)
                    ex4 = eex[:, 1, u].rearrange("p (k o i) -> p k o i", k=K6, o=OH)
                    x24 = (
                        x2_sb[:, tt * KC : tt * KC + K6 * C]
                        .rearrange("p (k i) -> p k i", k=K6)
                        .unsqueeze(2)
                        .broadcast_to([TT, K6, OH, C])
                    )
                    nc.vector.tensor_mul(ex4, e4, x24)

                # --- quad-wide tail (all DVE ops bf16 @2x) ---
                # num k-sum tree: 6 ex slabs + x6.
                exu = eex[:, 1].rearrange(
                    "p s (x k q) -> p (s x) k q", x=3, k=2
                )  # [TT, 12(u,x), 2, 512]
                t1n = tpool.tile([TT, QU, 3, SLAB], bf16, tag="t1n")
                t1nv = t1n[:].rearrange("p u x q -> p (u x) q")
                nc.vector.tensor_add(t1nv, exu[:, :, 0], exu[:, :, 1])
                t2n = spool.tile([TT, QU, SLAB], bf16, tag="t2n")
                nc.vector.tensor_add(t2n[:], t1n[:, :, 0], t1n[:, :, 1])
                t3n = spool.tile([TT, QU, SLAB], bf16, tag="t3n")
                x6b = (
                    x2_sb[:, pp * QU * KC : (pp + 1) * QU * KC]
                    .rearrange("p (u j i) -> p u j i", u=QU, j=K)[:, :, K6]
                    .unsqueeze(2)
                    .broadcast_to([TT, QU, OH, C])
                )
                nc.vector.tensor_add(
                    t3n[:].rearrange("p u (o i) -> p u o i", o=OH),
                    t1n[:, :, 2].rearrange("p u (o i) -> p u o i", o=OH),
                    x6b,
                )
                dn2 = spool.tile([TT, QU, SLAB], bf16, tag="dn2")
                nc.vector.tensor_add(dn2[:], t2n[:], t3n[:])

                # den k-sum tree (no +1: it rides the Ln bias).
                eu = eex[:, 0].rearrange("p s (x k q) -> p (s x) k q", x=3, k=2)
                t1d = tpool.tile([TT, QU, 3, SLAB], bf16, tag="t1d")
                t1dv = t1d[:].rearrange("p u x q -> p (u x) q")
                nc.vector.tensor_add(t1dv, eu[:, :, 0], eu[:, :, 1])
                t2d = spool.tile([TT, QU, SLAB], bf16, tag="t2d")
                nc.vector.tensor_add(t2d[:], t1d[:, :, 0], t1d[:, :, 1])
                denb = hpool.tile([TT, QU, SLAB], bf16, tag="denb")
                nc.vector.tensor_add(denb[:], t1d[:, :, 2], t2d[:])

                # 1/den on ScalarE: exp(-ln(den + 1)).
                lden = hpool.tile([TT, QU, SLAB], bf16, tag="lden")
                nc.scalar.activation(lden[:], denb[:], AF.Ln, bias=1.0)
                rr = hpool.tile([TT, QU, SLAB], bf16, tag="rr")
                nc.scalar.activation(rr[:], lden[:], AF.Exp, scale=-1.0)

                # y[t,o] = sum_i num * (1/den): mul, halve over i, reduce.
                y1 = spool.tile([TT, QU, SLAB], bf16, tag="y1")
                nc.vector.tensor_mul(y1[:], dn2[:], rr[:])
                yh = spool.tile([TT, QU * OH, C // 2], bf16, tag="yh")
                y1h = y1[:].rearrange("p u (o h i) -> p (u o) h i", o=OH, h=2)
                nc.vector.tensor_add(yh[:], y1h[:, :, 0], y1h[:, :, 1])
                nc.vector.tensor_reduce(
                    y_sb[:, pp * QU * OH : (pp + 1) * QU * OH],
                    yh[:],
                    axis=mybir.AxisListType.X,
                    op=ADD,
                )

                if (pp + 1) % 2 == 0 or pp == nt // QU - 1:
                    g0 = (pp // 2) * 2 * QU * OH
                    nc.sync.dma_start(
                        y_d[:, g0 : (pp + 1) * QU * OH],
                        y_sb[:, g0 : (pp + 1) * QU * OH],
                    )

    # Both Exp and Ln live in the natural_log_exp_and_others act table, but
    # the table-load pass picks the first set containing each function,
    # alternating tables (a 1.3us reload per switch). Trim exp/ln from every
    # other set (indices into act_info.json are preserved) so one load serves
    # the whole kernel.
    from concourse.hw_specs import get_activation_tables

    tabs = get_activation_tables(nc.m.arch)
    for name, s in tabs.items():
        if name != "natural_log_exp_and_others":
            s.discard(mybir.ActivationFunctionType.Exp)
            s.discard(mybir.ActivationFunctionType.Ln)

    nc.compile()
    return nc


def _prep_inputs(x, W, b):
    """Host-side scatter: per-core input dicts (layout + the k=6 shift)."""
    import ml_dtypes

    bf = ml_dtypes.bfloat16
    f8 = ml_dtypes.float8_e4m3fn
    scale = np.float32(WSC / np.sqrt(K))
    halves = []
    for h in range(2):
        Wh = W[h * OH * C * K : (h + 1) * OH * C * K]  # rows (o, i, k)
        W5 = Wh.reshape(OH, C, K, C, K)  # (o, i, k, c, j)
        # softmax shift: subtract the k=6 tap, drop it
        Ws = W5[:, :, :K6] - W5[:, :, K6 : K6 + 1]
        # rows (j,c) -> j*32+c ; cols (k,o,i) -> k*512 + o*32 + i, k<6
        Wp = Ws.transpose(4, 3, 2, 0, 1).reshape(K * C, FREE) * scale
        bh = b[h * OH * C * K : (h + 1) * OH * C * K].reshape(OH, C, K)
        bs = (bh[:, :, :K6] - bh[:, :, K6 : K6 + 1]) * scale
        bs = bs.transpose(2, 0, 1).reshape(FREE)
        w8 = np.zeros((CD1, 2, FREE), dtype=np.float32)
        w8[:, 0] = Wp[:CD1]
        w8[:96, 1] = Wp[CD1 : CD1 + 96]
        w8[96, 1] = bs
        halves.append(w8.astype(f8))

    t_len = x.shape[-1]
    nt = t_len // TT
    xs = []
    for bi in range(B):
        xp = np.zeros((C, t_len + 2 * PAD), dtype=np.float32)
        xp[:, PAD : PAD + t_len] = x[bi]
        x1 = np.empty((K * C, t_len), dtype=np.float32)
        for j in range(K):
            x1[j * C : (j + 1) * C] = xp[:, j : j + t_len]
        x18 = np.zeros((CD1, 2, t_len), dtype=np.float32)
        x18[:, 0] = x1[:CD1]
        x18[:96, 1] = x1[CD1:KC]
        x18[96, 1] = 1.0
        # x2f[tp, tt*KC + j*C + c] = x[c, tt*TT + tp + j - 3]
        x2 = np.ascontiguousarray(
            x1.reshape(K, C, nt, TT).transpose(3, 2, 0, 1).reshape(TT, nt * KC)
        )
        xs.append((x18.astype(f8), x2.astype(bf)))

    in_maps = []
    for core in range(8):
        bi, h = divmod(core, 2)
        x18, x2 = xs[bi]
        in_maps.append({"x1f8": x18, "w8": halves[h], "x2f": x2})
    return in_maps


def _assemble(results, t_len):
    """Gather per-core [TT, nt*OH] outputs into [B, O_FULL, t_len]."""
    nt = t_len // TT
    y = np.empty((B, O_FULL, t_len), dtype=np.float32)
    for core, res in enumerate(results):
        bi, h = divmod(core, 2)
        arr = res["yout"].reshape(TT, nt, OH)  # [tp, tt, o]
        y[bi, h * OH : (h + 1) * OH, :] = arr.transpose(2, 1, 0).reshape(OH, t_len)
    return y


def _run(x, W, b, trace=False, trace_cores=None):
    from concourse.bass_utils import run_bass_kernel_spmd
    from concourse.bass_interp import get_hw_module

    t_len = x.shape[-1]
    key = ("prog", t_len)
    if key not in _prog_cache:
        nc = _build(t_len)
        nc.m = get_hw_module(nc.m)
        _prog_cache[key] = nc
    nc = _prog_cache[key]

    in_maps = _prep_inputs(x, W, b)
    res = run_bass_kernel_spmd(
        nc,
        in_maps,
        core_ids=list(range(8)),
        trace=trace,
        trace_cores=trace_cores,
    )
    return _assemble(res.results, t_len), res


def kernel(x, W, b):
    y, _ = _run(np.asarray(x), np.asarray(W), np.asarray(b))
    return y
